# revision 14
# baseline (speedup 1.0000x reference)
"""Multi-head attention + residual + LayerNorm on 8 trn2 NeuronCores.

Sharding: core c -> (batch b = c//4, head-group g = c%4). Each core computes
4 heads (256 output dims) of attention for its batch over the full sequence,
plus its 256-dim slice of the residual+LayerNorm output. LayerNorm row
statistics are completed with a tiny AllReduce over each 4-core batch group.
"""

import os
import numpy as np
import ml_dtypes

import concourse.bass as bass
import concourse.bacc as bacc
import concourse.mybir as mybir
import concourse.tile as tile
from concourse.bass_utils import run_bass_kernel_spmd

# Problem shape (hardcoded per contract)
B, S, D, H, DH = 2, 2048, 1024, 16, 64
EPS = 1e-12
NCORES = 8
GROUPS = 4          # head-groups (cores per batch)
DSL = D // GROUPS   # 256 output dims per core
NHL = H // GROUPS   # 4 local heads per core
P = 128
KT_N = S // P       # 16 q pos-tiles
SKC = 1280          # compacted+padded key positions (unmasked ~1024 of 2048)
KC_N = SKC // P     # 10 key tiles
DIN_C = D // P      # 8 contraction chunks
DT2 = DSL // P      # 2 dout tiles for Q/K
QH_N = 2            # q halves
QHS = S // QH_N     # 1024
MASK_BIAS = -30.0   # exp(-30) ~ 1e-13: numerically identical to -10000 mask

F32 = mybir.dt.float32
BF16 = mybir.dt.bfloat16
I32 = mybir.dt.int32
npbf16 = ml_dtypes.bfloat16

AX_X = mybir.AxisListType.X
OP = mybir.AluOpType
AF = mybir.ActivationFunctionType


def _install_ntff_hook():
    """Provide antenv.axon_hooks if the image lacks it (enables NTFF timing)."""
    import sys
    import types
    try:
        from antenv.axon_hooks import get_axon_ntff_profile_hook  # noqa: F401
        return
    except ImportError:
        pass
    try:
        import antenv
        from trn_agent_boot.trn_boot import _ntff_profile_via_ctypes
        mod = types.ModuleType("antenv.axon_hooks")
        state = {"hook": _ntff_profile_via_ctypes("/opt/axon/libaxon_pjrt.so")}
        mod.set_axon_ntff_profile_hook = lambda h: state.update(hook=h)
        mod.get_axon_ntff_profile_hook = lambda: state["hook"]
        sys.modules["antenv.axon_hooks"] = mod
        antenv.axon_hooks = mod
    except Exception:
        pass


_cached_nc = None
last_results = None  # BassKernelResults of the most recent run (for test harness)


def _emit(tc):
    nc = tc.nc

    # ---------------- I/O ----------------
    qT_d = nc.dram_tensor("qT", [QH_N, D, QHS], BF16, kind="ExternalInput")
    kT_d = nc.dram_tensor("kT", [D, SKC], BF16, kind="ExternalInput")
    vT_d = nc.dram_tensor("vT", [D, SKC], BF16, kind="ExternalInput")
    wq_d = nc.dram_tensor("wq", [D, DSL], BF16, kind="ExternalInput")
    wk_d = nc.dram_tensor("wk", [D, DSL], BF16, kind="ExternalInput")
    wv_d = nc.dram_tensor("wv", [D, DSL], BF16, kind="ExternalInput")
    bqp_d = nc.dram_tensor("bqp", [P, NHL], F32, kind="ExternalInput")
    bkp_d = nc.dram_tensor("bkp", [P, NHL], F32, kind="ExternalInput")
    bvr_d = nc.dram_tensor("bvr", [P, DSL], F32, kind="ExternalInput")
    gr_d = nc.dram_tensor("gr", [P, DSL], F32, kind="ExternalInput")
    br_d = nc.dram_tensor("br", [P, DSL], F32, kind="ExternalInput")
    msk_d = nc.dram_tensor("mask", [P, KC_N], I32, kind="ExternalInput")
    qres_d = nc.dram_tensor("qres", [S, DSL], F32, kind="ExternalInput")
    out_d = nc.dram_tensor("out", [S, DSL], F32, kind="ExternalOutput")

    with (
        tc.tile_pool(name="const", bufs=1) as constp,
        tc.tile_pool(name="weights", bufs=1) as wpool,
        tc.tile_pool(name="vt", bufs=1) as vtp,
        tc.tile_pool(name="qkstream", bufs=4) as qksp,
        tc.tile_pool(name="qtkt", bufs=1) as qtkp,
        tc.tile_pool(name="vaug", bufs=1) as vaugp,
        tc.tile_pool(name="es", bufs=24) as esp,
        tc.tile_pool(name="hsb", bufs=1) as hp,
        tc.tile_pool(name="small", bufs=4) as smallp,
        tc.tile_pool(name="lnio", bufs=3) as lniop,
        tc.tile_pool(name="ps", bufs=3, space="PSUM") as psp,
        tc.tile_pool(name="dram", bufs=1, space="DRAM") as dramp,
    ):
        # ---------------- small constants ----------------
        bq_sb = constp.tile([P, NHL], F32, tag="bq")
        bk_sb = constp.tile([P, NHL], F32, tag="bk")
        nc.sync.dma_start(bq_sb[:], bqp_d[:])
        nc.sync.dma_start(bk_sb[:], bkp_d[:])
        mski = constp.tile([P, KC_N], I32, tag="mski")
        nc.sync.dma_start(mski[:], msk_d[:])
        mskf = constp.tile([P, KC_N], F32, tag="mskf")
        nc.vector.tensor_copy(mskf[:], mski[:])
        mbias = constp.tile([P, KC_N], F32, tag="mbias")
        nc.vector.tensor_scalar(mbias[:], mskf[:], -MASK_BIAS, MASK_BIAS, OP.mult, OP.add)

        # per-head padded projections: rows 0:64 = head data, rows 64:128 = 0
        qt_pad = qtkp.tile([P, NHL, S], BF16, tag="qt")
        kt_pad = qtkp.tile([P, NHL, SKC], BF16, tag="kt")
        nc.vector.memset(qt_pad[64:128, :, :], 0.0)
        nc.vector.memset(kt_pad[64:128, :, :], 0.0)

        # ---------------- Q^T / K^T projections ----------------
        # qt_pad[p, h, s]: rows 0:64 = head h of (x @ W + b)^T, rows 64:128 = 0
        def load_w(w_sb, w_d, eng):
            for c in range(DIN_C):
                eng.dma_start(w_sb[:, DSL * c:DSL * (c + 1)],
                              w_d[P * c:P * (c + 1), :])

        def project(src_d, w_sb, b_sb, dst_pad, sw, sname, eng, gids):
            for gi in gids:
                off = 1024 * gi
                width = min(1024, sw - off)
                prj = [psp.tile([P, QHS], F32, tag="mm",
                                name=f"prj_{sname}{gi}_{t}") for t in range(DT2)]
                for c in range(DIN_C):
                    xch = qksp.tile([P, QHS], BF16, tag="xch",
                                    name=f"xch_{sname}{gi}_{c}")
                    if len(src_d.shape) == 3:
                        eng.dma_start(xch[:, 0:width],
                                      src_d[gi, P * c:P * (c + 1), :])
                    else:
                        eng.dma_start(xch[:, 0:width],
                                      src_d[P * c:P * (c + 1), off:off + width])
                    for t in range(DT2):
                        for m in range(0, width, 512):
                            mw = min(512, width - m)
                            nc.tensor.matmul(
                                prj[t][:, m:m + mw],
                                lhsT=w_sb[:, DSL * c + P * t:DSL * c + P * (t + 1)],
                                rhs=xch[:, m:m + mw],
                                start=(c == 0), stop=(c == DIN_C - 1),
                            )
                for t in range(DT2):
                    for hh in range(2):
                        h = 2 * t + hh
                        nc.vector.tensor_scalar_add(
                            dst_pad[0:DH, h, off:off + width],
                            prj[t][DH * hh:DH * (hh + 1), 0:width],
                            b_sb[0:DH, h:h + 1])

        wq_sb = wpool.tile([P, DIN_C * DSL], BF16, tag="wq")
        load_w(wq_sb, wq_d, nc.sync)
        project(qT_d, wq_sb, bq_sb, qt_pad, S, "q", nc.sync, [0])
        wk_sb = wpool.tile([P, DIN_C * DSL], BF16, tag="wk")
        load_w(wk_sb, wk_d, nc.scalar)
        project(kT_d, wk_sb, bk_sb, kt_pad, SKC, "k", nc.scalar,
                list(range((SKC + 1023) // 1024)))

        # ---------------- attention helpers ----------------
        h_sb = hp.tile([P, KT_N, DSL], F32, tag="hsb")
        ssum = constp.tile([P, KT_N], F32, tag="ssum")
        ssq = constp.tile([P, KT_N], F32, tag="ssq")
        xacc = constp.tile([P, KT_N, NHL], F32, tag="xacc")

        def emit_qres(qh):
            qrs = []
            for j in range(8):
                pt = 8 * qh + j
                qr = lniop.tile([P, DSL], F32, tag="qr", bufs=9, name=f"qr{pt}")
                nc.scalar.dma_start(qr[:], qres_d[P * pt:P * (pt + 1), :])
                qrs.append(qr)
            return qrs

        def scores_exp(qh, h):
            es_tiles = []
            for kt in range(KC_N):
                s_ps = psp.tile([P, QHS], F32, tag="mm", name=f"sps{qh}_{h}_{kt}")
                for m in range(2):
                    nc.tensor.matmul(
                        s_ps[:, 512 * m:512 * (m + 1)],
                        lhsT=kt_pad[:, h, P * kt:P * (kt + 1)],
                        rhs=qt_pad[:, h,
                                   QHS * qh + 512 * m:QHS * qh + 512 * (m + 1)],
                        start=True, stop=True,
                    )
                es = esp.tile([P, QHS], BF16, tag="es", name=f"es{qh}_{h}_{kt}")
                nc.scalar.activation(es[:], s_ps[:], AF.Exp,
                                     bias=mbias[:, kt:kt + 1], scale=0.125)
                es_tiles.append(es)
            return es_tiles

        def attnv_drain(qh, h, es_tiles, qrs):
            for hj in range(2):
                H_ps = psp.tile([P, 4 * (DH + 1)], F32, tag="hps", bufs=2,
                                name=f"hps{qh}_{h}_{hj}")
                for kt in range(KC_N):
                    for jj in range(4):
                        j = 4 * hj + jj
                        nc.tensor.matmul(
                            H_ps[:, 65 * jj:65 * jj + DH + 1],
                            lhsT=es_tiles[kt][:, P * j:P * (j + 1)],
                            rhs=vaug[:, kt, (DH + 1) * h:(DH + 1) * (h + 1)],
                            start=(kt == 0 and jj == 0),
                            stop=(kt == KC_N - 1 and jj == 3),
                        )
                rec4 = smallp.tile([P, 4, 1], F32, tag="rec",
                                   name=f"rec{qh}_{h}_{hj}")
                nc.vector.reciprocal(
                    rec4[:],
                    H_ps[:].rearrange("p (s x) -> p s x", x=DH + 1)[:, :, DH:DH + 1])
                for jj in range(4):
                    j = 4 * hj + jj
                    pt = 8 * qh + j
                    nc.vector.scalar_tensor_tensor(
                        out=h_sb[:, pt, DH * h:DH * (h + 1)],
                        in0=H_ps[:, 65 * jj:65 * jj + DH],
                        scalar=rec4[:, jj, :],
                        in1=qrs[j][:, DH * h:DH * (h + 1)],
                        op0=OP.mult, op1=OP.add,
                        accum_out=xacc[:, pt, h:h + 1])

        def stats_ar_norm(qh):
            for j in range(8):
                pt = 8 * qh + j
                nc.vector.reduce_sum(ssum[:, pt:pt + 1], xacc[:, pt, :], axis=AX_X)
                sq = lniop.tile([P, DSL], F32, tag="sqs", name=f"sq{pt}")
                if qh == 0:
                    nc.vector.tensor_tensor(sq[:], h_sb[:, pt, :], h_sb[:, pt, :],
                                            OP.mult)
                    nc.vector.reduce_sum(ssq[:, pt:pt + 1], sq[:], axis=AX_X)
                else:
                    nc.scalar.activation(sq[:], h_sb[:, pt, :], AF.Square,
                                         accum_out=ssq[:, pt:pt + 1])

            stin_t = dramp.tile([P, 16], F32, tag=f"stin{qh}", name=f"stin{qh}")
            stout_t = dramp.tile([P, 16], F32, tag=f"stout{qh}", name=f"stout{qh}")
            nc.sync.dma_start(stin_t[:, 0:8], ssum[:, 8 * qh:8 * (qh + 1)])
            nc.sync.dma_start(stin_t[:, 8:16], ssq[:, 8 * qh:8 * (qh + 1)])
            nc.gpsimd.collective_compute(
                "AllReduce", OP.add,
                replica_groups=[[0, 1, 2, 3], [4, 5, 6, 7]],
                ins=[stin_t.opt()], outs=[stout_t.opt()],
            )
            stats = constp.tile([P, 16], F32, tag=f"stats{qh}", name=f"stats{qh}")
            nc.sync.dma_start(stats[:], stout_t[:])

            u_all = constp.tile([P, 8], F32, tag=f"u{qh}", name=f"u{qh}")
            v_all = constp.tile([P, 8], F32, tag=f"v{qh}", name=f"v{qh}")
            u2 = constp.tile([P, 8], F32, tag=f"u2{qh}", name=f"u2{qh}")
            rstd = constp.tile([P, 8], F32, tag=f"rstd{qh}", name=f"rstd{qh}")
            nc.vector.tensor_scalar_mul(u_all[:], stats[:, 0:8], 1.0 / D)
            nc.vector.tensor_scalar_mul(v_all[:], stats[:, 8:16], 1.0 / D)
            nc.vector.tensor_tensor(u2[:], u_all[:], u_all[:], OP.mult)
            nc.vector.tensor_tensor(v_all[:], v_all[:], u2[:], OP.subtract)
            nc.vector.tensor_scalar_add(v_all[:], v_all[:], EPS)
            lnv = constp.tile([P, 8], F32, tag=f"lnv{qh}", name=f"lnv{qh}")
            nc.scalar.activation(lnv[:], v_all[:], AF.Ln)
            nc.scalar.activation(rstd[:], lnv[:], AF.Exp, scale=-0.5)

            for j in range(8):
                pt = 8 * qh + j
                o = lniop.tile([P, DSL], F32, tag="o", name=f"o{pt}")
                nc.vector.scalar_tensor_tensor(
                    out=o[:], in0=h_sb[:, pt, :], scalar=u_all[:, j:j + 1],
                    in1=gr_sb[:], op0=OP.subtract, op1=OP.mult)
                nc.vector.scalar_tensor_tensor(
                    out=o[:], in0=o[:], scalar=rstd[:, j:j + 1],
                    in1=br_sb[:], op0=OP.mult, op1=OP.add)
                nc.sync.dma_start(out_d[P * pt:P * (pt + 1), :], o[:])

        # ---------------- schedule ----------------
        qrs0 = emit_qres(0)
        es00 = scores_exp(0, 0)

        # V projection emitted here: its matmuls hide under (0,0)'s exp phase
        bvr_sb = constp.tile([P, DSL], F32, tag="bvr")
        nc.sync.dma_start(bvr_sb[:], bvr_d[:])
        wv_sb = wpool.tile([P, DIN_C * DSL], BF16, tag="wv")
        load_w(wv_sb, wv_d, nc.scalar)
        vt_sb = vtp.tile([P, DIN_C, SKC], BF16, tag="vt")
        for c in range(DIN_C):
            nc.scalar.dma_start(vt_sb[:, c, :], vT_d[P * c:P * (c + 1), :])
        vaug = vaugp.tile([P, KC_N, NHL * (DH + 1)], BF16, tag="vaug")
        nc.vector.memset(
            vaug[:].rearrange("p t (h x) -> p t h x", h=NHL)[:, :, :, DH:DH + 1], 1.0)
        for pt in range(KC_N):
            v_ps = psp.tile([P, QHS], F32, tag="mm", name=f"vps{pt}")
            for c in range(DIN_C):
                nc.tensor.matmul(
                    v_ps[:, 0:DSL],
                    lhsT=vt_sb[:, c, P * pt:P * (pt + 1)],
                    rhs=wv_sb[:, DSL * c:DSL * (c + 1)],
                    start=(c == 0), stop=(c == DIN_C - 1),
                )
            nc.vector.tensor_tensor(
                vaug[:, pt, :].rearrange("p (h x) -> p h x", h=NHL)[:, :, 0:DH],
                v_ps[:, 0:DSL].rearrange("p (h x) -> p h x", h=NHL),
                bvr_sb[:].rearrange("p (h x) -> p h x", h=NHL),
                OP.add,
            )
        gr_sb = constp.tile([P, DSL], F32, tag="gr")
        br_sb = constp.tile([P, DSL], F32, tag="br")
        nc.sync.dma_start(gr_sb[:], gr_d[:])
        nc.sync.dma_start(br_sb[:], br_d[:])

        attnv_drain(0, 0, es00, qrs0)
        for h in range(1, NHL):
            attnv_drain(0, h, scores_exp(0, h), qrs0)
        stats_ar_norm(0)

        # second q-half projection hides under qh=0 attention
        project(qT_d, wq_sb, bq_sb, qt_pad, S, "q", nc.sync, [1])
        qrs1 = emit_qres(1)
        for h in range(NHL):
            attnv_drain(1, h, scores_exp(1, h), qrs1)
        stats_ar_norm(1)


def build(skc=SKC):
    global SKC, KC_N
    SKC, KC_N = skc, skc // P
    nc = bacc.Bacc("TRN2", target_bir_lowering=False, debug=False,
                   num_devices=NCORES)
    with tile.TileContext(nc) as tc:
        _emit(tc)
    nc.compile()
    return nc


def _shard(inputs, skc=None):
    skc = skc or SKC
    q = np.ascontiguousarray(np.asarray(inputs["query"], dtype=np.float32))
    k = np.ascontiguousarray(np.asarray(inputs["key"], dtype=np.float32))
    v = np.ascontiguousarray(np.asarray(inputs["value"], dtype=np.float32))
    mask = np.ascontiguousarray(np.asarray(inputs["mask"], dtype=np.int32))
    Wq = np.asarray(inputs["Wq"], dtype=np.float32)
    Wk = np.asarray(inputs["Wk"], dtype=np.float32)
    Wv = np.asarray(inputs["Wv"], dtype=np.float32)
    bq = np.asarray(inputs["bq"], dtype=np.float32)
    bk = np.asarray(inputs["bk"], dtype=np.float32)
    bv = np.asarray(inputs["bv"], dtype=np.float32)
    gamma = np.asarray(inputs["gamma"], dtype=np.float32)
    beta = np.asarray(inputs["beta"], dtype=np.float32)

    qT, kT, vT, mc = [], [], [], []
    for b in range(B):
        idx = np.nonzero(mask[b])[0]
        sc = len(idx)
        assert sc <= skc, f"unmasked keys {sc} > {skc}"
        kc = np.zeros((skc, D), np.float32)
        vc = np.zeros((skc, D), np.float32)
        kc[:sc] = k[b][idx]
        vc[:sc] = v[b][idx]
        m = np.zeros(skc, np.int32)
        m[:sc] = 1
        qt_full = q[b].T.astype(npbf16)
        qT.append(np.ascontiguousarray(
            np.stack([qt_full[:, QHS * g:QHS * (g + 1)] for g in range(QH_N)])))
        kT.append(np.ascontiguousarray(kc.T).astype(npbf16))
        vT.append(np.ascontiguousarray(vc.T).astype(npbf16))
        mc.append(np.ascontiguousarray(m.reshape(skc // P, P).T))

    def bias_ph(bvec):
        out = np.zeros((P, NHL), np.float32)
        out[0:DH] = bvec.reshape(NHL, DH).T
        return out

    in_maps = []
    for c in range(NCORES):
        b, g = divmod(c, GROUPS)
        dsl = slice(DSL * g, DSL * (g + 1))
        in_maps.append({
            "qT": qT[b],
            "kT": kT[b],
            "vT": vT[b],
            "wq": np.ascontiguousarray(Wq[:, dsl]).astype(npbf16),
            "wk": np.ascontiguousarray(Wk[:, dsl]).astype(npbf16),
            "wv": np.ascontiguousarray(Wv[:, dsl]).astype(npbf16),
            "bqp": bias_ph(bq[dsl]),
            "bkp": bias_ph(bk[dsl]),
            "bvr": np.ascontiguousarray(np.broadcast_to(bv[dsl], (P, DSL))),
            "gr": np.ascontiguousarray(np.broadcast_to(gamma[dsl], (P, DSL))),
            "br": np.ascontiguousarray(np.broadcast_to(beta[dsl], (P, DSL))),
            "mask": mc[b],
            "qres": np.ascontiguousarray(q[b][:, dsl]),
        })
    return in_maps


_nc_cache = {}


def kernel(**inputs):
    global last_results
    mask = np.asarray(inputs["mask"])
    max_sc = int(max((mask[b] != 0).sum() for b in range(B)))
    skc = max(P, -(-max_sc // P) * P)
    if skc not in _nc_cache:
        _nc_cache[skc] = build(skc)
    nc = _nc_cache[skc]
    in_maps = _shard(inputs, skc)
    trace = bool(int(os.environ.get("KERNEL_TRACE", "0")))
    if trace:
        _install_ntff_hook()
        import concourse.bass_utils as _bu
        _bu.upload_artifacts = lambda tmpdir: tmpdir
    res = run_bass_kernel_spmd(nc, in_maps, core_ids=list(range(NCORES)),
                               trace=trace)
    last_results = res
    out = np.empty((B, S, D), dtype=np.float32)
    for c in range(NCORES):
        b, g = divmod(c, GROUPS)
        out[b, :, DSL * g:DSL * (g + 1)] = res.results[c]["out"]
    return out


if __name__ == "__main__":
    nc = build()
    print("build ok; instructions:", sum(1 for _ in nc.m.functions[0].basicblocks for _ in _.instructions) if hasattr(nc.m.functions[0], "basicblocks") else "?")


# revision 15
# speedup vs baseline: 1.0040x; 1.0040x over previous
"""Multi-head attention + residual + LayerNorm on 8 trn2 NeuronCores.

Sharding: core c -> (batch b = c//4, head-group g = c%4). Each core computes
4 heads (256 output dims) of attention for its batch over the full sequence,
plus its 256-dim slice of the residual+LayerNorm output. LayerNorm row
statistics are completed with a tiny AllReduce over each 4-core batch group.
"""

import os
import numpy as np
import ml_dtypes

import concourse.bass as bass
import concourse.bacc as bacc
import concourse.mybir as mybir
import concourse.tile as tile
from concourse.bass_utils import run_bass_kernel_spmd

# Problem shape (hardcoded per contract)
B, S, D, H, DH = 2, 2048, 1024, 16, 64
EPS = 1e-12
NCORES = 8
GROUPS = 4          # head-groups (cores per batch)
DSL = D // GROUPS   # 256 output dims per core
NHL = H // GROUPS   # 4 local heads per core
P = 128
KT_N = S // P       # 16 q pos-tiles
SKC = 1280          # compacted+padded key positions (unmasked ~1024 of 2048)
KC_N = SKC // P     # 10 key tiles
DIN_C = D // P      # 8 contraction chunks
DT2 = DSL // P      # 2 dout tiles for Q/K
QH_N = 2            # q halves
QHS = S // QH_N     # 1024
MASK_BIAS = -30.0   # exp(-30) ~ 1e-13: numerically identical to -10000 mask

F32 = mybir.dt.float32
BF16 = mybir.dt.bfloat16
I32 = mybir.dt.int32
npbf16 = ml_dtypes.bfloat16

AX_X = mybir.AxisListType.X
OP = mybir.AluOpType
AF = mybir.ActivationFunctionType


def _install_ntff_hook():
    """Provide antenv.axon_hooks if the image lacks it (enables NTFF timing)."""
    import sys
    import types
    try:
        from antenv.axon_hooks import get_axon_ntff_profile_hook  # noqa: F401
        return
    except ImportError:
        pass
    try:
        import antenv
        from trn_agent_boot.trn_boot import _ntff_profile_via_ctypes
        mod = types.ModuleType("antenv.axon_hooks")
        state = {"hook": _ntff_profile_via_ctypes("/opt/axon/libaxon_pjrt.so")}
        mod.set_axon_ntff_profile_hook = lambda h: state.update(hook=h)
        mod.get_axon_ntff_profile_hook = lambda: state["hook"]
        sys.modules["antenv.axon_hooks"] = mod
        antenv.axon_hooks = mod
    except Exception:
        pass


_cached_nc = None
last_results = None  # BassKernelResults of the most recent run (for test harness)


def _emit(tc):
    nc = tc.nc

    # ---------------- I/O ----------------
    qT_d = nc.dram_tensor("qT", [QH_N, D, QHS], BF16, kind="ExternalInput")
    kT_d = nc.dram_tensor("kT", [D, SKC], BF16, kind="ExternalInput")
    vT_d = nc.dram_tensor("vT", [D, SKC], BF16, kind="ExternalInput")
    wq_d = nc.dram_tensor("wq", [D, DSL], BF16, kind="ExternalInput")
    wk_d = nc.dram_tensor("wk", [D, DSL], BF16, kind="ExternalInput")
    wv_d = nc.dram_tensor("wv", [D, DSL], BF16, kind="ExternalInput")
    bqp_d = nc.dram_tensor("bqp", [P, NHL], F32, kind="ExternalInput")
    bkp_d = nc.dram_tensor("bkp", [P, NHL], F32, kind="ExternalInput")
    bvr_d = nc.dram_tensor("bvr", [P, DSL], F32, kind="ExternalInput")
    gr_d = nc.dram_tensor("gr", [P, DSL], F32, kind="ExternalInput")
    br_d = nc.dram_tensor("br", [P, DSL], F32, kind="ExternalInput")
    msk_d = nc.dram_tensor("mask", [P, KC_N], I32, kind="ExternalInput")
    qres_d = nc.dram_tensor("qres", [S, DSL], F32, kind="ExternalInput")
    out_d = nc.dram_tensor("out", [S, DSL], F32, kind="ExternalOutput")

    with (
        tc.tile_pool(name="const", bufs=1) as constp,
        tc.tile_pool(name="weights", bufs=1) as wpool,
        tc.tile_pool(name="vt", bufs=1) as vtp,
        tc.tile_pool(name="qkstream", bufs=4) as qksp,
        tc.tile_pool(name="qtkt", bufs=1) as qtkp,
        tc.tile_pool(name="vaug", bufs=1) as vaugp,
        tc.tile_pool(name="es", bufs=28) as esp,
        tc.tile_pool(name="hsb", bufs=1) as hp,
        tc.tile_pool(name="small", bufs=4) as smallp,
        tc.tile_pool(name="lnio", bufs=3) as lniop,
        tc.tile_pool(name="ps", bufs=3, space="PSUM") as psp,
        tc.tile_pool(name="dram", bufs=1, space="DRAM") as dramp,
    ):
        # ---------------- small constants ----------------
        bq_sb = constp.tile([P, NHL], F32, tag="bq")
        bk_sb = constp.tile([P, NHL], F32, tag="bk")
        nc.sync.dma_start(bq_sb[:], bqp_d[:])
        nc.sync.dma_start(bk_sb[:], bkp_d[:])
        mski = constp.tile([P, KC_N], I32, tag="mski")
        nc.sync.dma_start(mski[:], msk_d[:])
        mskf = constp.tile([P, KC_N], F32, tag="mskf")
        nc.vector.tensor_copy(mskf[:], mski[:])
        mbias = constp.tile([P, KC_N], F32, tag="mbias")
        nc.vector.tensor_scalar(mbias[:], mskf[:], -MASK_BIAS, MASK_BIAS, OP.mult, OP.add)

        # per-head padded projections: rows 0:64 = head data, rows 64:128 = 0
        qt_pad = qtkp.tile([P, NHL, S], BF16, tag="qt")
        kt_pad = qtkp.tile([P, NHL, SKC], BF16, tag="kt")
        nc.vector.memset(qt_pad[64:128, :, :], 0.0)
        nc.vector.memset(kt_pad[64:128, :, :], 0.0)

        # ---------------- Q^T / K^T projections ----------------
        # qt_pad[p, h, s]: rows 0:64 = head h of (x @ W + b)^T, rows 64:128 = 0
        def load_w(w_sb, w_d, eng):
            for c in range(DIN_C):
                eng.dma_start(w_sb[:, DSL * c:DSL * (c + 1)],
                              w_d[P * c:P * (c + 1), :])

        def project(src_d, w_sb, b_sb, dst_pad, sw, sname, eng, gids):
            for gi in gids:
                off = 1024 * gi
                width = min(1024, sw - off)
                prj = [psp.tile([P, QHS], F32, tag="mm",
                                name=f"prj_{sname}{gi}_{t}") for t in range(DT2)]
                for c in range(DIN_C):
                    xch = qksp.tile([P, QHS], BF16, tag="xch",
                                    name=f"xch_{sname}{gi}_{c}")
                    if len(src_d.shape) == 3:
                        eng.dma_start(xch[:, 0:width],
                                      src_d[gi, P * c:P * (c + 1), :])
                    else:
                        eng.dma_start(xch[:, 0:width],
                                      src_d[P * c:P * (c + 1), off:off + width])
                    for t in range(DT2):
                        for m in range(0, width, 512):
                            mw = min(512, width - m)
                            nc.tensor.matmul(
                                prj[t][:, m:m + mw],
                                lhsT=w_sb[:, DSL * c + P * t:DSL * c + P * (t + 1)],
                                rhs=xch[:, m:m + mw],
                                start=(c == 0), stop=(c == DIN_C - 1),
                            )
                for t in range(DT2):
                    for hh in range(2):
                        h = 2 * t + hh
                        nc.vector.tensor_scalar_add(
                            dst_pad[0:DH, h, off:off + width],
                            prj[t][DH * hh:DH * (hh + 1), 0:width],
                            b_sb[0:DH, h:h + 1])

        wq_sb = wpool.tile([P, DIN_C * DSL], BF16, tag="wq")
        load_w(wq_sb, wq_d, nc.sync)
        project(qT_d, wq_sb, bq_sb, qt_pad, S, "q", nc.sync, [0])
        wk_sb = wpool.tile([P, DIN_C * DSL], BF16, tag="wk")
        load_w(wk_sb, wk_d, nc.scalar)
        project(kT_d, wk_sb, bk_sb, kt_pad, SKC, "k", nc.scalar,
                list(range((SKC + 1023) // 1024)))

        # ---------------- attention helpers ----------------
        h_sb = hp.tile([P, KT_N, DSL], F32, tag="hsb")
        ssum = constp.tile([P, KT_N], F32, tag="ssum")
        ssq = constp.tile([P, KT_N], F32, tag="ssq")
        xacc = constp.tile([P, KT_N, NHL], F32, tag="xacc")

        def emit_qres(qh):
            qrs = []
            for j in range(8):
                pt = 8 * qh + j
                qr = lniop.tile([P, DSL], F32, tag="qr", bufs=9, name=f"qr{pt}")
                nc.scalar.dma_start(qr[:], qres_d[P * pt:P * (pt + 1), :])
                qrs.append(qr)
            return qrs

        def scores_exp(qh, h):
            es_tiles = []
            for kt in range(KC_N):
                s_ps = psp.tile([P, QHS], F32, tag="mm", name=f"sps{qh}_{h}_{kt}")
                for m in range(2):
                    nc.tensor.matmul(
                        s_ps[:, 512 * m:512 * (m + 1)],
                        lhsT=kt_pad[:, h, P * kt:P * (kt + 1)],
                        rhs=qt_pad[:, h,
                                   QHS * qh + 512 * m:QHS * qh + 512 * (m + 1)],
                        start=True, stop=True,
                    )
                es = esp.tile([P, QHS], BF16, tag="es", name=f"es{qh}_{h}_{kt}")
                nc.scalar.activation(es[:], s_ps[:], AF.Exp,
                                     bias=mbias[:, kt:kt + 1], scale=0.125)
                es_tiles.append(es)
            return es_tiles

        def attnv_drain(qh, h, es_tiles, qrs):
            for hj in range(2):
                H_ps = psp.tile([P, 4 * (DH + 1)], F32, tag="hps", bufs=2,
                                name=f"hps{qh}_{h}_{hj}")
                for kt in range(KC_N):
                    for jj in range(4):
                        j = 4 * hj + jj
                        nc.tensor.matmul(
                            H_ps[:, 65 * jj:65 * jj + DH + 1],
                            lhsT=es_tiles[kt][:, P * j:P * (j + 1)],
                            rhs=vaug[:, kt, (DH + 1) * h:(DH + 1) * (h + 1)],
                            start=(kt == 0 and jj == 0),
                            stop=(kt == KC_N - 1 and jj == 3),
                        )
                rec4 = smallp.tile([P, 4, 1], F32, tag="rec",
                                   name=f"rec{qh}_{h}_{hj}")
                nc.vector.reciprocal(
                    rec4[:],
                    H_ps[:].rearrange("p (s x) -> p s x", x=DH + 1)[:, :, DH:DH + 1])
                for jj in range(4):
                    j = 4 * hj + jj
                    pt = 8 * qh + j
                    nc.vector.scalar_tensor_tensor(
                        out=h_sb[:, pt, DH * h:DH * (h + 1)],
                        in0=H_ps[:, 65 * jj:65 * jj + DH],
                        scalar=rec4[:, jj, :],
                        in1=qrs[j][:, DH * h:DH * (h + 1)],
                        op0=OP.mult, op1=OP.add,
                        accum_out=xacc[:, pt, h:h + 1])

        def stats_ar_norm(qh):
            for j in range(8):
                pt = 8 * qh + j
                nc.vector.reduce_sum(ssum[:, pt:pt + 1], xacc[:, pt, :], axis=AX_X)
                sq = lniop.tile([P, DSL], F32, tag="sqs", name=f"sq{pt}")
                if qh == 0:
                    nc.vector.tensor_tensor(sq[:], h_sb[:, pt, :], h_sb[:, pt, :],
                                            OP.mult)
                    nc.vector.reduce_sum(ssq[:, pt:pt + 1], sq[:], axis=AX_X)
                else:
                    nc.scalar.activation(sq[:], h_sb[:, pt, :], AF.Square,
                                         accum_out=ssq[:, pt:pt + 1])

            stin_t = dramp.tile([P, 16], F32, tag=f"stin{qh}", name=f"stin{qh}")
            stout_t = dramp.tile([P, 16], F32, tag=f"stout{qh}", name=f"stout{qh}")
            nc.sync.dma_start(stin_t[:, 0:8], ssum[:, 8 * qh:8 * (qh + 1)])
            nc.sync.dma_start(stin_t[:, 8:16], ssq[:, 8 * qh:8 * (qh + 1)])
            nc.gpsimd.collective_compute(
                "AllReduce", OP.add,
                replica_groups=[[0, 1, 2, 3], [4, 5, 6, 7]],
                ins=[stin_t.opt()], outs=[stout_t.opt()],
            )
            stats = constp.tile([P, 16], F32, tag=f"stats{qh}", name=f"stats{qh}")
            nc.sync.dma_start(stats[:], stout_t[:])

            u_all = constp.tile([P, 8], F32, tag=f"u{qh}", name=f"u{qh}")
            v_all = constp.tile([P, 8], F32, tag=f"v{qh}", name=f"v{qh}")
            u2 = constp.tile([P, 8], F32, tag=f"u2{qh}", name=f"u2{qh}")
            rstd = constp.tile([P, 8], F32, tag=f"rstd{qh}", name=f"rstd{qh}")
            nc.vector.tensor_scalar_mul(u_all[:], stats[:, 0:8], 1.0 / D)
            nc.vector.tensor_scalar_mul(v_all[:], stats[:, 8:16], 1.0 / D)
            nc.vector.tensor_tensor(u2[:], u_all[:], u_all[:], OP.mult)
            nc.vector.tensor_tensor(v_all[:], v_all[:], u2[:], OP.subtract)
            nc.vector.tensor_scalar_add(v_all[:], v_all[:], EPS)
            lnv = constp.tile([P, 8], F32, tag=f"lnv{qh}", name=f"lnv{qh}")
            nc.scalar.activation(lnv[:], v_all[:], AF.Ln)
            nc.scalar.activation(rstd[:], lnv[:], AF.Exp, scale=-0.5)

            for j in range(8):
                pt = 8 * qh + j
                o = lniop.tile([P, DSL], F32, tag="o", name=f"o{pt}")
                nc.vector.scalar_tensor_tensor(
                    out=o[:], in0=h_sb[:, pt, :], scalar=u_all[:, j:j + 1],
                    in1=gr_sb[:], op0=OP.subtract, op1=OP.mult)
                nc.vector.scalar_tensor_tensor(
                    out=o[:], in0=o[:], scalar=rstd[:, j:j + 1],
                    in1=br_sb[:], op0=OP.mult, op1=OP.add)
                nc.sync.dma_start(out_d[P * pt:P * (pt + 1), :], o[:])

        # ---------------- schedule ----------------
        qrs0 = emit_qres(0)
        es00 = scores_exp(0, 0)

        # V projection emitted here: its matmuls hide under (0,0)'s exp phase
        bvr_sb = constp.tile([P, DSL], F32, tag="bvr")
        nc.sync.dma_start(bvr_sb[:], bvr_d[:])
        wv_sb = wpool.tile([P, DIN_C * DSL], BF16, tag="wv")
        load_w(wv_sb, wv_d, nc.scalar)
        vt_sb = vtp.tile([P, DIN_C, SKC], BF16, tag="vt")
        for c in range(DIN_C):
            nc.scalar.dma_start(vt_sb[:, c, :], vT_d[P * c:P * (c + 1), :])
        vaug = vaugp.tile([P, KC_N, NHL * (DH + 1)], BF16, tag="vaug")
        nc.vector.memset(
            vaug[:].rearrange("p t (h x) -> p t h x", h=NHL)[:, :, :, DH:DH + 1], 1.0)
        for pt in range(KC_N):
            v_ps = psp.tile([P, QHS], F32, tag="mm", name=f"vps{pt}")
            for c in range(DIN_C):
                nc.tensor.matmul(
                    v_ps[:, 0:DSL],
                    lhsT=vt_sb[:, c, P * pt:P * (pt + 1)],
                    rhs=wv_sb[:, DSL * c:DSL * (c + 1)],
                    start=(c == 0), stop=(c == DIN_C - 1),
                )
            nc.vector.tensor_tensor(
                vaug[:, pt, :].rearrange("p (h x) -> p h x", h=NHL)[:, :, 0:DH],
                v_ps[:, 0:DSL].rearrange("p (h x) -> p h x", h=NHL),
                bvr_sb[:].rearrange("p (h x) -> p h x", h=NHL),
                OP.add,
            )
        gr_sb = constp.tile([P, DSL], F32, tag="gr")
        br_sb = constp.tile([P, DSL], F32, tag="br")
        nc.sync.dma_start(gr_sb[:], gr_d[:])
        nc.sync.dma_start(br_sb[:], br_d[:])

        attnv_drain(0, 0, es00, qrs0)
        for h in range(1, NHL):
            attnv_drain(0, h, scores_exp(0, h), qrs0)

        # second q-half projection: emitted before stats so its input DMAs
        # sit ahead of qh0's AR-gated output DMAs in the sync queue
        project(qT_d, wq_sb, bq_sb, qt_pad, S, "q", nc.sync, [1])
        qrs1 = emit_qres(1)
        stats_ar_norm(0)
        for h in range(NHL):
            attnv_drain(1, h, scores_exp(1, h), qrs1)
        stats_ar_norm(1)


def build(skc=SKC):
    global SKC, KC_N
    SKC, KC_N = skc, skc // P
    nc = bacc.Bacc("TRN2", target_bir_lowering=False, debug=False,
                   num_devices=NCORES)
    with tile.TileContext(nc) as tc:
        _emit(tc)
    nc.compile()
    return nc


def _shard(inputs, skc=None):
    skc = skc or SKC
    q = np.ascontiguousarray(np.asarray(inputs["query"], dtype=np.float32))
    k = np.ascontiguousarray(np.asarray(inputs["key"], dtype=np.float32))
    v = np.ascontiguousarray(np.asarray(inputs["value"], dtype=np.float32))
    mask = np.ascontiguousarray(np.asarray(inputs["mask"], dtype=np.int32))
    Wq = np.asarray(inputs["Wq"], dtype=np.float32)
    Wk = np.asarray(inputs["Wk"], dtype=np.float32)
    Wv = np.asarray(inputs["Wv"], dtype=np.float32)
    bq = np.asarray(inputs["bq"], dtype=np.float32)
    bk = np.asarray(inputs["bk"], dtype=np.float32)
    bv = np.asarray(inputs["bv"], dtype=np.float32)
    gamma = np.asarray(inputs["gamma"], dtype=np.float32)
    beta = np.asarray(inputs["beta"], dtype=np.float32)

    qT, kT, vT, mc = [], [], [], []
    for b in range(B):
        idx = np.nonzero(mask[b])[0]
        sc = len(idx)
        assert sc <= skc, f"unmasked keys {sc} > {skc}"
        kc = np.zeros((skc, D), np.float32)
        vc = np.zeros((skc, D), np.float32)
        kc[:sc] = k[b][idx]
        vc[:sc] = v[b][idx]
        m = np.zeros(skc, np.int32)
        m[:sc] = 1
        qt_full = q[b].T.astype(npbf16)
        qT.append(np.ascontiguousarray(
            np.stack([qt_full[:, QHS * g:QHS * (g + 1)] for g in range(QH_N)])))
        kT.append(np.ascontiguousarray(kc.T).astype(npbf16))
        vT.append(np.ascontiguousarray(vc.T).astype(npbf16))
        mc.append(np.ascontiguousarray(m.reshape(skc // P, P).T))

    def bias_ph(bvec):
        out = np.zeros((P, NHL), np.float32)
        out[0:DH] = bvec.reshape(NHL, DH).T
        return out

    in_maps = []
    for c in range(NCORES):
        b, g = divmod(c, GROUPS)
        dsl = slice(DSL * g, DSL * (g + 1))
        in_maps.append({
            "qT": qT[b],
            "kT": kT[b],
            "vT": vT[b],
            "wq": np.ascontiguousarray(Wq[:, dsl]).astype(npbf16),
            "wk": np.ascontiguousarray(Wk[:, dsl]).astype(npbf16),
            "wv": np.ascontiguousarray(Wv[:, dsl]).astype(npbf16),
            "bqp": bias_ph(bq[dsl]),
            "bkp": bias_ph(bk[dsl]),
            "bvr": np.ascontiguousarray(np.broadcast_to(bv[dsl], (P, DSL))),
            "gr": np.ascontiguousarray(np.broadcast_to(gamma[dsl], (P, DSL))),
            "br": np.ascontiguousarray(np.broadcast_to(beta[dsl], (P, DSL))),
            "mask": mc[b],
            "qres": np.ascontiguousarray(q[b][:, dsl]),
        })
    return in_maps


_nc_cache = {}


def kernel(**inputs):
    global last_results
    mask = np.asarray(inputs["mask"])
    max_sc = int(max((mask[b] != 0).sum() for b in range(B)))
    skc = max(P, -(-max_sc // P) * P)
    if skc not in _nc_cache:
        _nc_cache[skc] = build(skc)
    nc = _nc_cache[skc]
    in_maps = _shard(inputs, skc)
    trace = bool(int(os.environ.get("KERNEL_TRACE", "0")))
    if trace:
        _install_ntff_hook()
        import concourse.bass_utils as _bu
        _bu.upload_artifacts = lambda tmpdir: tmpdir
    res = run_bass_kernel_spmd(nc, in_maps, core_ids=list(range(NCORES)),
                               trace=trace)
    last_results = res
    out = np.empty((B, S, D), dtype=np.float32)
    for c in range(NCORES):
        b, g = divmod(c, GROUPS)
        out[b, :, DSL * g:DSL * (g + 1)] = res.results[c]["out"]
    return out


if __name__ == "__main__":
    nc = build()
    print("build ok; instructions:", sum(1 for _ in nc.m.functions[0].basicblocks for _ in _.instructions) if hasattr(nc.m.functions[0], "basicblocks") else "?")


# revision 22
# speedup vs baseline: 1.0884x; 1.0841x over previous
"""Multi-head attention + residual + LayerNorm on 8 trn2 NeuronCores.

Sharding: core c -> (batch b = c//4, head-group g = c%4). Each core computes
4 heads (256 output dims) of attention for its batch over the full sequence,
plus its 256-dim slice of the residual+LayerNorm output. LayerNorm row
statistics are completed with a tiny AllReduce over each 4-core batch group.
"""

import os
import numpy as np
import ml_dtypes

import concourse.bass as bass
import concourse.bacc as bacc
import concourse.mybir as mybir
import concourse.tile as tile
from concourse.bass_utils import run_bass_kernel_spmd

# Problem shape (hardcoded per contract)
B, S, D, H, DH = 2, 2048, 1024, 16, 64
EPS = 1e-12
NCORES = 8
GROUPS = 4          # head-groups (cores per batch)
DSL = D // GROUPS   # 256 output dims per core
NHL = H // GROUPS   # 4 local heads per core
P = 128
KT_N = S // P       # 16 q pos-tiles
SKC = 1280          # compacted+padded key positions (unmasked ~1024 of 2048)
KC_N = SKC // P     # 10 key tiles
DIN_C = D // P      # 8 contraction chunks
DT2 = DSL // P      # 2 dout tiles for Q/K
QH_N = 2            # q halves
QHS = S // QH_N     # 1024
MASK_BIAS = -30.0   # exp(-30) ~ 1e-13: numerically identical to -10000 mask

F32 = mybir.dt.float32
BF16 = mybir.dt.bfloat16
I32 = mybir.dt.int32
npbf16 = ml_dtypes.bfloat16

AX_X = mybir.AxisListType.X
OP = mybir.AluOpType
AF = mybir.ActivationFunctionType


def _install_ntff_hook():
    """Provide antenv.axon_hooks if the image lacks it (enables NTFF timing)."""
    import sys
    import types
    try:
        from antenv.axon_hooks import get_axon_ntff_profile_hook  # noqa: F401
        return
    except ImportError:
        pass
    try:
        import antenv
        from trn_agent_boot.trn_boot import _ntff_profile_via_ctypes
        mod = types.ModuleType("antenv.axon_hooks")
        state = {"hook": _ntff_profile_via_ctypes("/opt/axon/libaxon_pjrt.so")}
        mod.set_axon_ntff_profile_hook = lambda h: state.update(hook=h)
        mod.get_axon_ntff_profile_hook = lambda: state["hook"]
        sys.modules["antenv.axon_hooks"] = mod
        antenv.axon_hooks = mod
    except Exception:
        pass


_cached_nc = None
last_results = None  # BassKernelResults of the most recent run (for test harness)


def _emit(tc):
    nc = tc.nc

    # ---------------- I/O ----------------
    qT_d = nc.dram_tensor("qT", [QH_N, D, QHS], BF16, kind="ExternalInput")
    kT_d = nc.dram_tensor("kT", [D, SKC], BF16, kind="ExternalInput")
    vT_d = nc.dram_tensor("vT", [D, SKC], BF16, kind="ExternalInput")
    wq_d = nc.dram_tensor("wq", [D, DSL], BF16, kind="ExternalInput")
    wk_d = nc.dram_tensor("wk", [D, DSL], BF16, kind="ExternalInput")
    wv_d = nc.dram_tensor("wv", [D, DSL], BF16, kind="ExternalInput")
    bqp_d = nc.dram_tensor("bqp", [P, NHL], F32, kind="ExternalInput")
    bkp_d = nc.dram_tensor("bkp", [P, NHL], F32, kind="ExternalInput")
    bvr_d = nc.dram_tensor("bvr", [P, DSL], F32, kind="ExternalInput")
    gr_d = nc.dram_tensor("gr", [P, DSL], F32, kind="ExternalInput")
    br_d = nc.dram_tensor("br", [P, DSL], F32, kind="ExternalInput")
    msk_d = nc.dram_tensor("mask", [P, KC_N], I32, kind="ExternalInput")
    qres_d = nc.dram_tensor("qres", [S, DSL], F32, kind="ExternalInput")
    out_d = nc.dram_tensor("out", [S, DSL], F32, kind="ExternalOutput")

    with (
        tc.tile_pool(name="const", bufs=1) as constp,
        tc.tile_pool(name="weights", bufs=1) as wpool,
        tc.tile_pool(name="vt", bufs=1) as vtp,
        tc.tile_pool(name="qkstream", bufs=4) as qksp,
        tc.tile_pool(name="qtkt", bufs=1) as qtkp,
        tc.tile_pool(name="vaug", bufs=1) as vaugp,
        tc.tile_pool(name="es", bufs=28) as esp,
        tc.tile_pool(name="hsb", bufs=1) as hp,
        tc.tile_pool(name="small", bufs=4) as smallp,
        tc.tile_pool(name="lnio", bufs=3) as lniop,
        tc.tile_pool(name="ps", bufs=3, space="PSUM") as psp,
        tc.tile_pool(name="dram", bufs=1, space="DRAM") as dramp,
    ):
        # ---------------- small constants ----------------
        bq_sb = constp.tile([P, NHL], F32, tag="bq")
        bk_sb = constp.tile([P, NHL], F32, tag="bk")
        nc.sync.dma_start(bq_sb[:], bqp_d[:])
        nc.sync.dma_start(bk_sb[:], bkp_d[:])
        mski = constp.tile([P, KC_N], I32, tag="mski")
        nc.sync.dma_start(mski[:], msk_d[:])
        mskf = constp.tile([P, KC_N], F32, tag="mskf")
        nc.vector.tensor_copy(mskf[:], mski[:])
        mbias = constp.tile([P, KC_N], F32, tag="mbias")
        nc.vector.tensor_scalar(mbias[:], mskf[:], -MASK_BIAS, MASK_BIAS, OP.mult, OP.add)

        # per-head padded projections: rows 0:64 = head data, rows 64:128 = 0
        qt_sb = qtkp.tile([P, NHL, S], BF16, tag="qt")
        kt_sb = qtkp.tile([P, NHL, SKC], BF16, tag="kt")
        nc.vector.memset(qt_sb[64:128, :, :], 0.0)
        nc.vector.memset(kt_sb[64:128, :, :], 0.0)

        # ---------------- Q^T / K^T projections ----------------
        # qt_pad[p, h, s]: rows 0:64 = head h of (x @ W + b)^T, rows 64:128 = 0
        def load_w(w_sb, w_d, eng):
            for c in range(DIN_C):
                eng.dma_start(w_sb[:, DSL * c:DSL * (c + 1)],
                              w_d[P * c:P * (c + 1), :])

        def project(src_d, w_sb, b_sb, dst_pad, sw, sname, eng, gids):
            for gi in gids:
                off = 1024 * gi
                width = min(1024, sw - off)
                prj = [psp.tile([P, QHS], F32, tag="mm", bufs=3,
                                name=f"prj_{sname}{gi}_{t}") for t in range(DT2)]
                for c in range(DIN_C):
                    xch = qksp.tile([P, QHS], BF16, tag="xch",
                                    name=f"xch_{sname}{gi}_{c}")
                    if len(src_d.shape) == 3:
                        eng.dma_start(xch[:, 0:width],
                                      src_d[gi, P * c:P * (c + 1), :])
                    else:
                        eng.dma_start(xch[:, 0:width],
                                      src_d[P * c:P * (c + 1), off:off + width])
                    for t in range(DT2):
                        for m in range(0, width, 512):
                            mw = min(512, width - m)
                            nc.tensor.matmul(
                                prj[t][:, m:m + mw],
                                lhsT=w_sb[:, DSL * c + P * t:DSL * c + P * (t + 1)],
                                rhs=xch[:, m:m + mw],
                                start=(c == 0), stop=(c == DIN_C - 1),
                            )
                for t in range(DT2):
                    for hh in range(2):
                        h = 2 * t + hh
                        nc.vector.tensor_scalar_add(
                            dst_pad[0:DH, h, off:off + width],
                            prj[t][DH * hh:DH * (hh + 1), 0:width],
                            b_sb[0:DH, h:h + 1])

        wq_sb = wpool.tile([P, DIN_C * DSL], BF16, tag="wq")
        load_w(wq_sb, wq_d, nc.sync)
        project(qT_d, wq_sb, bq_sb, qt_sb, S, "q", nc.gpsimd, [0])
        wk_sb = wpool.tile([P, DIN_C * DSL], BF16, tag="wk")
        load_w(wk_sb, wk_d, nc.scalar)
        project(kT_d, wk_sb, bk_sb, kt_sb, SKC, "k", nc.scalar,
                list(range((SKC + 1023) // 1024)))

        # ---------------- attention helpers ----------------
        h_sb = hp.tile([P, KT_N, DSL], F32, tag="hsb")
        ssum = constp.tile([P, KT_N], F32, tag="ssum")
        ssq = constp.tile([P, KT_N], F32, tag="ssq")
        xacc = constp.tile([P, KT_N, NHL], F32, tag="xacc")

        def emit_qres(qh):
            qrs = []
            for j in range(8):
                pt = 8 * qh + j
                qr = lniop.tile([P, DSL], F32, tag="qr", bufs=9, name=f"qr{pt}")
                nc.scalar.dma_start(qr[:], qres_d[P * pt:P * (pt + 1), :])
                qrs.append(qr)
            return qrs

        def scores_exp(qh, h):
            es_tiles = []
            for kt in range(KC_N):
                s_ps = psp.tile([P, QHS], F32, tag="mm", bufs=3,
                                name=f"sps{qh}_{h}_{kt}")
                for m in range(2):
                    nc.tensor.matmul(
                        s_ps[:, 512 * m:512 * (m + 1)],
                        lhsT=kt_sb[:, h, P * kt:P * (kt + 1)],
                        rhs=qt_sb[:, h,
                                  QHS * qh + 512 * m:QHS * qh + 512 * (m + 1)],
                        start=True, stop=True,
                    )
                es = esp.tile([P, QHS], BF16, tag="es", name=f"es{qh}_{h}_{kt}")
                nc.scalar.activation(es[:], s_ps[:], AF.Exp,
                                     bias=mbias[:, kt:kt + 1], scale=0.125)
                es_tiles.append(es)
            return es_tiles

        def attnv_drain(qh, h, es_tiles, qrs):
            for hj in range(2):
                H_ps = psp.tile([P, 4 * (DH + 1)], F32, tag="hps", bufs=2,
                                name=f"hps{qh}_{h}_{hj}")
                for kt in range(KC_N):
                    for jj in range(4):
                        j = 4 * hj + jj
                        nc.tensor.matmul(
                            H_ps[:, 65 * jj:65 * jj + DH + 1],
                            lhsT=es_tiles[kt][:, P * j:P * (j + 1)],
                            rhs=vaug[:, kt, (DH + 1) * h:(DH + 1) * (h + 1)],
                            start=(kt == 0 and jj == 0),
                            stop=(kt == KC_N - 1 and jj == 3),
                        )
                rec4 = smallp.tile([P, 4, 1], F32, tag="rec",
                                   name=f"rec{qh}_{h}_{hj}")
                nc.vector.reciprocal(
                    rec4[:],
                    H_ps[:].rearrange("p (s x) -> p s x", x=DH + 1)[:, :, DH:DH + 1])
                for jj in range(4):
                    j = 4 * hj + jj
                    pt = 8 * qh + j
                    nc.vector.scalar_tensor_tensor(
                        out=h_sb[:, pt, DH * h:DH * (h + 1)],
                        in0=H_ps[:, 65 * jj:65 * jj + DH],
                        scalar=rec4[:, jj, :],
                        in1=qrs[j][:, DH * h:DH * (h + 1)],
                        op0=OP.mult, op1=OP.add,
                        accum_out=xacc[:, pt, h:h + 1])

        def stats_ar_norm(qh):
            for j in range(8):
                pt = 8 * qh + j
                nc.vector.reduce_sum(ssum[:, pt:pt + 1], xacc[:, pt, :], axis=AX_X)
                sq = lniop.tile([P, DSL], F32, tag="sqs", name=f"sq{pt}")
                if qh == 0:
                    nc.vector.tensor_tensor(sq[:], h_sb[:, pt, :], h_sb[:, pt, :],
                                            OP.mult)
                    nc.vector.reduce_sum(ssq[:, pt:pt + 1], sq[:], axis=AX_X)
                else:
                    nc.scalar.activation(sq[:], h_sb[:, pt, :], AF.Square,
                                         accum_out=ssq[:, pt:pt + 1])

            stin_t = dramp.tile([P, 16], F32, tag=f"stin{qh}", name=f"stin{qh}")
            stout_t = dramp.tile([P, 16], F32, tag=f"stout{qh}", name=f"stout{qh}")
            nc.sync.dma_start(stin_t[:, 0:8], ssum[:, 8 * qh:8 * (qh + 1)])
            nc.sync.dma_start(stin_t[:, 8:16], ssq[:, 8 * qh:8 * (qh + 1)])
            nc.gpsimd.collective_compute(
                "AllReduce", OP.add,
                replica_groups=[[0, 1, 2, 3], [4, 5, 6, 7]],
                ins=[stin_t.opt()], outs=[stout_t.opt()],
            )
            stats = constp.tile([P, 16], F32, tag=f"stats{qh}", name=f"stats{qh}")
            nc.sync.dma_start(stats[:], stout_t[:])

            u_all = constp.tile([P, 8], F32, tag=f"u{qh}", name=f"u{qh}")
            v_all = constp.tile([P, 8], F32, tag=f"v{qh}", name=f"v{qh}")
            u2 = constp.tile([P, 8], F32, tag=f"u2{qh}", name=f"u2{qh}")
            rstd = constp.tile([P, 8], F32, tag=f"rstd{qh}", name=f"rstd{qh}")
            nc.vector.tensor_scalar_mul(u_all[:], stats[:, 0:8], 1.0 / D)
            nc.vector.tensor_scalar_mul(v_all[:], stats[:, 8:16], 1.0 / D)
            nc.vector.tensor_tensor(u2[:], u_all[:], u_all[:], OP.mult)
            nc.vector.tensor_tensor(v_all[:], v_all[:], u2[:], OP.subtract)
            nc.vector.tensor_scalar_add(v_all[:], v_all[:], EPS)
            lnv = constp.tile([P, 8], F32, tag=f"lnv{qh}", name=f"lnv{qh}")
            nc.scalar.activation(lnv[:], v_all[:], AF.Ln)
            nc.scalar.activation(rstd[:], lnv[:], AF.Exp, scale=-0.5)

            for j in range(8):
                pt = 8 * qh + j
                o = lniop.tile([P, DSL], F32, tag="o", name=f"o{pt}")
                nc.vector.scalar_tensor_tensor(
                    out=o[:], in0=h_sb[:, pt, :], scalar=u_all[:, j:j + 1],
                    in1=gr_sb[:], op0=OP.subtract, op1=OP.mult)
                nc.vector.scalar_tensor_tensor(
                    out=o[:], in0=o[:], scalar=rstd[:, j:j + 1],
                    in1=br_sb[:], op0=OP.mult, op1=OP.add)
                nc.sync.dma_start(out_d[P * pt:P * (pt + 1), :], o[:])

        # ---------------- schedule ----------------
        qrs0 = emit_qres(0)
        es00 = scores_exp(0, 0)
        # V projection emitted here: its matmuls hide under (0,0)'s exp phase
        bvr_sb = constp.tile([P, DSL], F32, tag="bvr")
        nc.sync.dma_start(bvr_sb[:], bvr_d[:])
        wv_sb = wpool.tile([P, DIN_C * DSL], BF16, tag="wv")
        load_w(wv_sb, wv_d, nc.scalar)
        vt_sb = vtp.tile([P, DIN_C, SKC], BF16, tag="vt")
        for c in range(DIN_C):
            nc.scalar.dma_start(vt_sb[:, c, :], vT_d[P * c:P * (c + 1), :])
        vaug = vaugp.tile([P, KC_N, NHL * (DH + 1)], BF16, tag="vaug")
        nc.vector.memset(
            vaug[:].rearrange("p t (h x) -> p t h x", h=NHL)[:, :, :, DH:DH + 1], 1.0)
        for pt in range(KC_N):
            v_ps = psp.tile([P, 4 * (DH + 1)], F32, tag="hps", bufs=2,
                            name=f"vps{pt}")
            for c in range(DIN_C):
                nc.tensor.matmul(
                    v_ps[:, 0:DSL],
                    lhsT=vt_sb[:, c, P * pt:P * (pt + 1)],
                    rhs=wv_sb[:, DSL * c:DSL * (c + 1)],
                    start=(c == 0), stop=(c == DIN_C - 1),
                )
            nc.vector.tensor_tensor(
                vaug[:, pt, :].rearrange("p (h x) -> p h x", h=NHL)[:, :, 0:DH],
                v_ps[:, 0:DSL].rearrange("p (h x) -> p h x", h=NHL),
                bvr_sb[:].rearrange("p (h x) -> p h x", h=NHL),
                OP.add,
            )
        gr_sb = constp.tile([P, DSL], F32, tag="gr")
        br_sb = constp.tile([P, DSL], F32, tag="br")
        nc.sync.dma_start(gr_sb[:], gr_d[:])
        nc.sync.dma_start(br_sb[:], br_d[:])

        # second q-half projection: early, hidden under the exp backlog
        project(qT_d, wq_sb, bq_sb, qt_sb, S, "q", nc.gpsimd, [1])
        qrs1 = emit_qres(1)
        es01 = scores_exp(0, 1)
        # second q-half projection: early, hidden under the exp backlog
        project(qT_d, wq_sb, bq_sb, qt_sb, S, "q", nc.gpsimd, [1])
        qrs1 = emit_qres(1)
        attnv_drain(0, 0, es00, qrs0)
        es02 = scores_exp(0, 2)
        attnv_drain(0, 1, es01, qrs0)
        es03 = scores_exp(0, 3)
        attnv_drain(0, 2, es02, qrs0)
        attnv_drain(0, 3, es03, qrs0)
        stats_ar_norm(0)
        for h in range(NHL):
            attnv_drain(1, h, scores_exp(1, h), qrs1)
        stats_ar_norm(1)


def build(skc=SKC):
    global SKC, KC_N
    SKC, KC_N = skc, skc // P
    nc = bacc.Bacc("TRN2", target_bir_lowering=False, debug=False,
                   num_devices=NCORES)
    with tile.TileContext(nc) as tc:
        _emit(tc)
    nc.compile()
    return nc


def _shard(inputs, skc=None):
    skc = skc or SKC
    q = np.ascontiguousarray(np.asarray(inputs["query"], dtype=np.float32))
    k = np.ascontiguousarray(np.asarray(inputs["key"], dtype=np.float32))
    v = np.ascontiguousarray(np.asarray(inputs["value"], dtype=np.float32))
    mask = np.ascontiguousarray(np.asarray(inputs["mask"], dtype=np.int32))
    Wq = np.asarray(inputs["Wq"], dtype=np.float32)
    Wk = np.asarray(inputs["Wk"], dtype=np.float32)
    Wv = np.asarray(inputs["Wv"], dtype=np.float32)
    bq = np.asarray(inputs["bq"], dtype=np.float32)
    bk = np.asarray(inputs["bk"], dtype=np.float32)
    bv = np.asarray(inputs["bv"], dtype=np.float32)
    gamma = np.asarray(inputs["gamma"], dtype=np.float32)
    beta = np.asarray(inputs["beta"], dtype=np.float32)

    qT, kT, vT, mc = [], [], [], []
    for b in range(B):
        idx = np.nonzero(mask[b])[0]
        sc = len(idx)
        assert sc <= skc, f"unmasked keys {sc} > {skc}"
        kc = np.zeros((skc, D), np.float32)
        vc = np.zeros((skc, D), np.float32)
        kc[:sc] = k[b][idx]
        vc[:sc] = v[b][idx]
        m = np.zeros(skc, np.int32)
        m[:sc] = 1
        qt_full = q[b].T.astype(npbf16)
        qT.append(np.ascontiguousarray(
            np.stack([qt_full[:, QHS * g:QHS * (g + 1)] for g in range(QH_N)])))
        kT.append(np.ascontiguousarray(kc.T).astype(npbf16))
        vT.append(np.ascontiguousarray(vc.T).astype(npbf16))
        mc.append(np.ascontiguousarray(m.reshape(skc // P, P).T))

    def bias_ph(bvec):
        out = np.zeros((P, NHL), np.float32)
        out[0:DH] = bvec.reshape(NHL, DH).T
        return out

    in_maps = []
    for c in range(NCORES):
        b, g = divmod(c, GROUPS)
        dsl = slice(DSL * g, DSL * (g + 1))
        in_maps.append({
            "qT": qT[b],
            "kT": kT[b],
            "vT": vT[b],
            "wq": np.ascontiguousarray(Wq[:, dsl]).astype(npbf16),
            "wk": np.ascontiguousarray(Wk[:, dsl]).astype(npbf16),
            "wv": np.ascontiguousarray(Wv[:, dsl]).astype(npbf16),
            "bqp": bias_ph(bq[dsl]),
            "bkp": bias_ph(bk[dsl]),
            "bvr": np.ascontiguousarray(np.broadcast_to(bv[dsl], (P, DSL))),
            "gr": np.ascontiguousarray(np.broadcast_to(gamma[dsl], (P, DSL))),
            "br": np.ascontiguousarray(np.broadcast_to(beta[dsl], (P, DSL))),
            "mask": mc[b],
            "qres": np.ascontiguousarray(q[b][:, dsl]),
        })
    return in_maps


_nc_cache = {}


def kernel(**inputs):
    global last_results
    mask = np.asarray(inputs["mask"])
    max_sc = int(max((mask[b] != 0).sum() for b in range(B)))
    skc = max(P, -(-max_sc // P) * P)
    if skc not in _nc_cache:
        _nc_cache[skc] = build(skc)
    nc = _nc_cache[skc]
    in_maps = _shard(inputs, skc)
    trace = bool(int(os.environ.get("KERNEL_TRACE", "0")))
    if trace:
        _install_ntff_hook()
        import concourse.bass_utils as _bu
        _bu.upload_artifacts = lambda tmpdir: tmpdir
    res = run_bass_kernel_spmd(nc, in_maps, core_ids=list(range(NCORES)),
                               trace=trace)
    last_results = res
    out = np.empty((B, S, D), dtype=np.float32)
    for c in range(NCORES):
        b, g = divmod(c, GROUPS)
        out[b, :, DSL * g:DSL * (g + 1)] = res.results[c]["out"]
    return out


if __name__ == "__main__":
    nc = build()
    print("build ok; instructions:", sum(1 for _ in nc.m.functions[0].basicblocks for _ in _.instructions) if hasattr(nc.m.functions[0], "basicblocks") else "?")


# revision 23
# speedup vs baseline: 1.2124x; 1.1139x over previous
"""Multi-head attention + residual + LayerNorm on 8 trn2 NeuronCores.

Sharding: core c -> (batch b = c//4, head-group g = c%4). Each core computes
4 heads (256 output dims) of attention for its batch over the full sequence,
plus its 256-dim slice of the residual+LayerNorm output. LayerNorm row
statistics are completed with a tiny AllReduce over each 4-core batch group.
"""

import os
import numpy as np
import ml_dtypes

import concourse.bass as bass
import concourse.bacc as bacc
import concourse.mybir as mybir
import concourse.tile as tile
from concourse.bass_utils import run_bass_kernel_spmd

# Problem shape (hardcoded per contract)
B, S, D, H, DH = 2, 2048, 1024, 16, 64
EPS = 1e-12
NCORES = 8
GROUPS = 4          # head-groups (cores per batch)
DSL = D // GROUPS   # 256 output dims per core
NHL = H // GROUPS   # 4 local heads per core
P = 128
KT_N = S // P       # 16 q pos-tiles
SKC = 1280          # compacted+padded key positions (unmasked ~1024 of 2048)
KC_N = SKC // P     # 10 key tiles
DIN_C = D // P      # 8 contraction chunks
DT2 = DSL // P      # 2 dout tiles for Q/K
QH_N = 2            # q halves
QHS = S // QH_N     # 1024
MASK_BIAS = -30.0   # exp(-30) ~ 1e-13: numerically identical to -10000 mask

F32 = mybir.dt.float32
BF16 = mybir.dt.bfloat16
I32 = mybir.dt.int32
npbf16 = ml_dtypes.bfloat16

AX_X = mybir.AxisListType.X
OP = mybir.AluOpType
AF = mybir.ActivationFunctionType


def _install_ntff_hook():
    """Provide antenv.axon_hooks if the image lacks it (enables NTFF timing)."""
    import sys
    import types
    try:
        from antenv.axon_hooks import get_axon_ntff_profile_hook  # noqa: F401
        return
    except ImportError:
        pass
    try:
        import antenv
        from trn_agent_boot.trn_boot import _ntff_profile_via_ctypes
        mod = types.ModuleType("antenv.axon_hooks")
        state = {"hook": _ntff_profile_via_ctypes("/opt/axon/libaxon_pjrt.so")}
        mod.set_axon_ntff_profile_hook = lambda h: state.update(hook=h)
        mod.get_axon_ntff_profile_hook = lambda: state["hook"]
        sys.modules["antenv.axon_hooks"] = mod
        antenv.axon_hooks = mod
    except Exception:
        pass


_cached_nc = None
last_results = None  # BassKernelResults of the most recent run (for test harness)


def _emit(tc):
    nc = tc.nc

    # ---------------- I/O ----------------
    qT_d = nc.dram_tensor("qT", [QH_N, D, QHS], BF16, kind="ExternalInput")
    kT_d = nc.dram_tensor("kT", [D, SKC], BF16, kind="ExternalInput")
    vT_d = nc.dram_tensor("vT", [D, SKC], BF16, kind="ExternalInput")
    wq_d = nc.dram_tensor("wq", [D, DSL], BF16, kind="ExternalInput")
    wk_d = nc.dram_tensor("wk", [D, DSL], BF16, kind="ExternalInput")
    wv_d = nc.dram_tensor("wv", [D, DSL], BF16, kind="ExternalInput")
    bqp_d = nc.dram_tensor("bqp", [P, NHL], F32, kind="ExternalInput")
    bkp_d = nc.dram_tensor("bkp", [P, NHL], F32, kind="ExternalInput")
    bvr_d = nc.dram_tensor("bvr", [P, DSL], F32, kind="ExternalInput")
    gr_d = nc.dram_tensor("gr", [P, DSL], F32, kind="ExternalInput")
    br_d = nc.dram_tensor("br", [P, DSL], F32, kind="ExternalInput")
    msk_d = nc.dram_tensor("mask", [P, KC_N], I32, kind="ExternalInput")
    qres_d = nc.dram_tensor("qres", [S, DSL], F32, kind="ExternalInput")
    out_d = nc.dram_tensor("out", [S, DSL], F32, kind="ExternalOutput")

    with (
        tc.tile_pool(name="const", bufs=1) as constp,
        tc.tile_pool(name="weights", bufs=1) as wpool,
        tc.tile_pool(name="vt", bufs=1) as vtp,
        tc.tile_pool(name="qkstream", bufs=4) as qksp,
        tc.tile_pool(name="qtkt", bufs=1) as qtkp,
        tc.tile_pool(name="vaug", bufs=1) as vaugp,
        tc.tile_pool(name="es", bufs=28) as esp,
        tc.tile_pool(name="hsb", bufs=1) as hp,
        tc.tile_pool(name="small", bufs=4) as smallp,
        tc.tile_pool(name="lnio", bufs=3) as lniop,
        tc.tile_pool(name="ps", bufs=3, space="PSUM") as psp,
        tc.tile_pool(name="dram", bufs=1, space="DRAM") as dramp,
    ):
        # ---------------- small constants ----------------
        bq_sb = constp.tile([P, NHL], F32, tag="bq")
        bk_sb = constp.tile([P, NHL], F32, tag="bk")
        nc.sync.dma_start(bq_sb[:], bqp_d[:])
        nc.sync.dma_start(bk_sb[:], bkp_d[:])
        mski = constp.tile([P, KC_N], I32, tag="mski")
        nc.sync.dma_start(mski[:], msk_d[:])
        mskf = constp.tile([P, KC_N], F32, tag="mskf")
        nc.vector.tensor_copy(mskf[:], mski[:])
        mbias = constp.tile([P, KC_N], F32, tag="mbias")
        nc.vector.tensor_scalar(mbias[:], mskf[:], -MASK_BIAS, MASK_BIAS, OP.mult, OP.add)

        # per-head padded projections: rows 0:64 = head data, rows 64:128 = 0
        qt_sb = qtkp.tile([P, NHL, S], BF16, tag="qt")
        kt_sb = qtkp.tile([P, NHL, SKC], BF16, tag="kt")
        nc.vector.memset(qt_sb[64:128, :, :], 0.0)
        nc.vector.memset(kt_sb[64:128, :, :], 0.0)

        # ---------------- Q^T / K^T projections ----------------
        # qt_pad[p, h, s]: rows 0:64 = head h of (x @ W + b)^T, rows 64:128 = 0
        def load_w(w_sb, w_d, eng):
            for c in range(DIN_C):
                eng.dma_start(w_sb[:, DSL * c:DSL * (c + 1)],
                              w_d[P * c:P * (c + 1), :])

        def project(src_d, w_sb, b_sb, dst_pad, sw, sname, eng, gids):
            for gi in gids:
                off = 1024 * gi
                width = min(1024, sw - off)
                prj = [psp.tile([P, QHS], F32, tag="mm", bufs=3,
                                name=f"prj_{sname}{gi}_{t}") for t in range(DT2)]
                for c in range(DIN_C):
                    xch = qksp.tile([P, QHS], BF16, tag="xch",
                                    name=f"xch_{sname}{gi}_{c}")
                    if len(src_d.shape) == 3:
                        eng.dma_start(xch[:, 0:width],
                                      src_d[gi, P * c:P * (c + 1), :])
                    else:
                        eng.dma_start(xch[:, 0:width],
                                      src_d[P * c:P * (c + 1), off:off + width])
                    for t in range(DT2):
                        for m in range(0, width, 512):
                            mw = min(512, width - m)
                            nc.tensor.matmul(
                                prj[t][:, m:m + mw],
                                lhsT=w_sb[:, DSL * c + P * t:DSL * c + P * (t + 1)],
                                rhs=xch[:, m:m + mw],
                                start=(c == 0), stop=(c == DIN_C - 1),
                            )
                for t in range(DT2):
                    for hh in range(2):
                        h = 2 * t + hh
                        nc.vector.tensor_scalar_add(
                            dst_pad[0:DH, h, off:off + width],
                            prj[t][DH * hh:DH * (hh + 1), 0:width],
                            b_sb[0:DH, h:h + 1])

        wq_sb = wpool.tile([P, DIN_C * DSL], BF16, tag="wq")
        load_w(wq_sb, wq_d, nc.sync)
        project(qT_d, wq_sb, bq_sb, qt_sb, S, "q", nc.gpsimd, [0])
        wk_sb = wpool.tile([P, DIN_C * DSL], BF16, tag="wk")
        load_w(wk_sb, wk_d, nc.scalar)
        project(kT_d, wk_sb, bk_sb, kt_sb, SKC, "k", nc.scalar,
                list(range((SKC + 1023) // 1024)))

        # ---------------- attention helpers ----------------
        h_sb = hp.tile([P, KT_N, DSL], F32, tag="hsb")
        ssum = constp.tile([P, KT_N], F32, tag="ssum")
        ssq = constp.tile([P, KT_N], F32, tag="ssq")
        xacc = constp.tile([P, KT_N, NHL], F32, tag="xacc")

        def emit_qres(qh):
            qrs = []
            for j in range(8):
                pt = 8 * qh + j
                qr = lniop.tile([P, DSL], F32, tag="qr", bufs=9, name=f"qr{pt}")
                nc.scalar.dma_start(qr[:], qres_d[P * pt:P * (pt + 1), :])
                qrs.append(qr)
            return qrs

        def scores_exp(qh, h):
            es_tiles = []
            for kt in range(KC_N):
                s_ps = psp.tile([P, QHS], F32, tag="mm", bufs=3,
                                name=f"sps{qh}_{h}_{kt}")
                for m in range(2):
                    nc.tensor.matmul(
                        s_ps[:, 512 * m:512 * (m + 1)],
                        lhsT=kt_sb[:, h, P * kt:P * (kt + 1)],
                        rhs=qt_sb[:, h,
                                  QHS * qh + 512 * m:QHS * qh + 512 * (m + 1)],
                        start=True, stop=True,
                    )
                es = esp.tile([P, QHS], BF16, tag="es", name=f"es{qh}_{h}_{kt}")
                nc.scalar.activation(es[:], s_ps[:], AF.Exp,
                                     bias=mbias[:, kt:kt + 1], scale=0.125)
                es_tiles.append(es)
            return es_tiles

        def attnv_drain(qh, h, es_tiles, qrs):
            for hj in range(2):
                H_ps = psp.tile([P, 4 * (DH + 1)], F32, tag="hps", bufs=2,
                                name=f"hps{qh}_{h}_{hj}")
                for kt in range(KC_N):
                    for jj in range(4):
                        j = 4 * hj + jj
                        nc.tensor.matmul(
                            H_ps[:, 65 * jj:65 * jj + DH + 1],
                            lhsT=es_tiles[kt][:, P * j:P * (j + 1)],
                            rhs=vaug[:, kt, (DH + 1) * h:(DH + 1) * (h + 1)],
                            start=(kt == 0 and jj == 0),
                            stop=(kt == KC_N - 1 and jj == 3),
                        )
                rec4 = smallp.tile([P, 4, 1], F32, tag="rec",
                                   name=f"rec{qh}_{h}_{hj}")
                nc.vector.reciprocal(
                    rec4[:],
                    H_ps[:].rearrange("p (s x) -> p s x", x=DH + 1)[:, :, DH:DH + 1])
                for jj in range(4):
                    j = 4 * hj + jj
                    pt = 8 * qh + j
                    nc.vector.scalar_tensor_tensor(
                        out=h_sb[:, pt, DH * h:DH * (h + 1)],
                        in0=H_ps[:, 65 * jj:65 * jj + DH],
                        scalar=rec4[:, jj, :],
                        in1=qrs[j][:, DH * h:DH * (h + 1)],
                        op0=OP.mult, op1=OP.add,
                        accum_out=xacc[:, pt, h:h + 1])

        def stats_ar_norm(qh):
            for j in range(8):
                pt = 8 * qh + j
                nc.vector.reduce_sum(ssum[:, pt:pt + 1], xacc[:, pt, :], axis=AX_X)
                sq = lniop.tile([P, DSL], F32, tag="sqs", name=f"sq{pt}")
                if qh == 0:
                    nc.vector.tensor_tensor(sq[:], h_sb[:, pt, :], h_sb[:, pt, :],
                                            OP.mult)
                    nc.vector.reduce_sum(ssq[:, pt:pt + 1], sq[:], axis=AX_X)
                else:
                    nc.scalar.activation(sq[:], h_sb[:, pt, :], AF.Square,
                                         accum_out=ssq[:, pt:pt + 1])

            stin_t = dramp.tile([P, 16], F32, tag=f"stin{qh}", name=f"stin{qh}")
            stout_t = dramp.tile([P, 16], F32, tag=f"stout{qh}", name=f"stout{qh}")
            nc.sync.dma_start(stin_t[:, 0:8], ssum[:, 8 * qh:8 * (qh + 1)])
            nc.sync.dma_start(stin_t[:, 8:16], ssq[:, 8 * qh:8 * (qh + 1)])
            nc.gpsimd.collective_compute(
                "AllReduce", OP.add,
                replica_groups=[[0, 1, 2, 3], [4, 5, 6, 7]],
                ins=[stin_t.opt()], outs=[stout_t.opt()],
            )
            stats = constp.tile([P, 16], F32, tag=f"stats{qh}", name=f"stats{qh}")
            nc.sync.dma_start(stats[:], stout_t[:])

            u_all = constp.tile([P, 8], F32, tag=f"u{qh}", name=f"u{qh}")
            v_all = constp.tile([P, 8], F32, tag=f"v{qh}", name=f"v{qh}")
            u2 = constp.tile([P, 8], F32, tag=f"u2{qh}", name=f"u2{qh}")
            rstd = constp.tile([P, 8], F32, tag=f"rstd{qh}", name=f"rstd{qh}")
            nc.vector.tensor_scalar_mul(u_all[:], stats[:, 0:8], 1.0 / D)
            nc.vector.tensor_scalar_mul(v_all[:], stats[:, 8:16], 1.0 / D)
            nc.vector.tensor_tensor(u2[:], u_all[:], u_all[:], OP.mult)
            nc.vector.tensor_tensor(v_all[:], v_all[:], u2[:], OP.subtract)
            nc.vector.tensor_scalar_add(v_all[:], v_all[:], EPS)
            lnv = constp.tile([P, 8], F32, tag=f"lnv{qh}", name=f"lnv{qh}")
            nc.scalar.activation(lnv[:], v_all[:], AF.Ln)
            nc.scalar.activation(rstd[:], lnv[:], AF.Exp, scale=-0.5)

            for j in range(8):
                pt = 8 * qh + j
                o = lniop.tile([P, DSL], F32, tag="o", name=f"o{pt}")
                nc.vector.scalar_tensor_tensor(
                    out=o[:], in0=h_sb[:, pt, :], scalar=u_all[:, j:j + 1],
                    in1=gr_sb[:], op0=OP.subtract, op1=OP.mult)
                nc.vector.scalar_tensor_tensor(
                    out=o[:], in0=o[:], scalar=rstd[:, j:j + 1],
                    in1=br_sb[:], op0=OP.mult, op1=OP.add)
                nc.sync.dma_start(out_d[P * pt:P * (pt + 1), :], o[:])

        # ---------------- schedule ----------------
        qrs0 = emit_qres(0)
        es00 = scores_exp(0, 0)
        # V projection emitted here: its matmuls hide under (0,0)'s exp phase
        bvr_sb = constp.tile([P, DSL], F32, tag="bvr")
        nc.sync.dma_start(bvr_sb[:], bvr_d[:])
        wv_sb = wpool.tile([P, DIN_C * DSL], BF16, tag="wv")
        load_w(wv_sb, wv_d, nc.scalar)
        vt_sb = vtp.tile([P, DIN_C, SKC], BF16, tag="vt")
        for c in range(DIN_C):
            nc.scalar.dma_start(vt_sb[:, c, :], vT_d[P * c:P * (c + 1), :])
        vaug = vaugp.tile([P, KC_N, NHL * (DH + 1)], BF16, tag="vaug")
        nc.vector.memset(
            vaug[:].rearrange("p t (h x) -> p t h x", h=NHL)[:, :, :, DH:DH + 1], 1.0)
        for pt in range(KC_N):
            v_ps = psp.tile([P, 4 * (DH + 1)], F32, tag="hps", bufs=2,
                            name=f"vps{pt}")
            for c in range(DIN_C):
                nc.tensor.matmul(
                    v_ps[:, 0:DSL],
                    lhsT=vt_sb[:, c, P * pt:P * (pt + 1)],
                    rhs=wv_sb[:, DSL * c:DSL * (c + 1)],
                    start=(c == 0), stop=(c == DIN_C - 1),
                )
            nc.vector.tensor_tensor(
                vaug[:, pt, :].rearrange("p (h x) -> p h x", h=NHL)[:, :, 0:DH],
                v_ps[:, 0:DSL].rearrange("p (h x) -> p h x", h=NHL),
                bvr_sb[:].rearrange("p (h x) -> p h x", h=NHL),
                OP.add,
            )
        gr_sb = constp.tile([P, DSL], F32, tag="gr")
        br_sb = constp.tile([P, DSL], F32, tag="br")
        nc.sync.dma_start(gr_sb[:], gr_d[:])
        nc.sync.dma_start(br_sb[:], br_d[:])

        es01 = scores_exp(0, 1)
        # second q-half projection: early, hidden under the exp backlog
        project(qT_d, wq_sb, bq_sb, qt_sb, S, "q", nc.gpsimd, [1])
        qrs1 = emit_qres(1)
        attnv_drain(0, 0, es00, qrs0)
        es02 = scores_exp(0, 2)
        attnv_drain(0, 1, es01, qrs0)
        es03 = scores_exp(0, 3)
        attnv_drain(0, 2, es02, qrs0)
        attnv_drain(0, 3, es03, qrs0)
        stats_ar_norm(0)
        for h in range(NHL):
            attnv_drain(1, h, scores_exp(1, h), qrs1)
        stats_ar_norm(1)


def build(skc=SKC):
    global SKC, KC_N
    SKC, KC_N = skc, skc // P
    nc = bacc.Bacc("TRN2", target_bir_lowering=False, debug=False,
                   num_devices=NCORES)
    with tile.TileContext(nc) as tc:
        _emit(tc)
    nc.compile()
    return nc


def _shard(inputs, skc=None):
    skc = skc or SKC
    q = np.ascontiguousarray(np.asarray(inputs["query"], dtype=np.float32))
    k = np.ascontiguousarray(np.asarray(inputs["key"], dtype=np.float32))
    v = np.ascontiguousarray(np.asarray(inputs["value"], dtype=np.float32))
    mask = np.ascontiguousarray(np.asarray(inputs["mask"], dtype=np.int32))
    Wq = np.asarray(inputs["Wq"], dtype=np.float32)
    Wk = np.asarray(inputs["Wk"], dtype=np.float32)
    Wv = np.asarray(inputs["Wv"], dtype=np.float32)
    bq = np.asarray(inputs["bq"], dtype=np.float32)
    bk = np.asarray(inputs["bk"], dtype=np.float32)
    bv = np.asarray(inputs["bv"], dtype=np.float32)
    gamma = np.asarray(inputs["gamma"], dtype=np.float32)
    beta = np.asarray(inputs["beta"], dtype=np.float32)

    qT, kT, vT, mc = [], [], [], []
    for b in range(B):
        idx = np.nonzero(mask[b])[0]
        sc = len(idx)
        assert sc <= skc, f"unmasked keys {sc} > {skc}"
        kc = np.zeros((skc, D), np.float32)
        vc = np.zeros((skc, D), np.float32)
        kc[:sc] = k[b][idx]
        vc[:sc] = v[b][idx]
        m = np.zeros(skc, np.int32)
        m[:sc] = 1
        qt_full = q[b].T.astype(npbf16)
        qT.append(np.ascontiguousarray(
            np.stack([qt_full[:, QHS * g:QHS * (g + 1)] for g in range(QH_N)])))
        kT.append(np.ascontiguousarray(kc.T).astype(npbf16))
        vT.append(np.ascontiguousarray(vc.T).astype(npbf16))
        mc.append(np.ascontiguousarray(m.reshape(skc // P, P).T))

    def bias_ph(bvec):
        out = np.zeros((P, NHL), np.float32)
        out[0:DH] = bvec.reshape(NHL, DH).T
        return out

    in_maps = []
    for c in range(NCORES):
        b, g = divmod(c, GROUPS)
        dsl = slice(DSL * g, DSL * (g + 1))
        in_maps.append({
            "qT": qT[b],
            "kT": kT[b],
            "vT": vT[b],
            "wq": np.ascontiguousarray(Wq[:, dsl]).astype(npbf16),
            "wk": np.ascontiguousarray(Wk[:, dsl]).astype(npbf16),
            "wv": np.ascontiguousarray(Wv[:, dsl]).astype(npbf16),
            "bqp": bias_ph(bq[dsl]),
            "bkp": bias_ph(bk[dsl]),
            "bvr": np.ascontiguousarray(np.broadcast_to(bv[dsl], (P, DSL))),
            "gr": np.ascontiguousarray(np.broadcast_to(gamma[dsl], (P, DSL))),
            "br": np.ascontiguousarray(np.broadcast_to(beta[dsl], (P, DSL))),
            "mask": mc[b],
            "qres": np.ascontiguousarray(q[b][:, dsl]),
        })
    return in_maps


_nc_cache = {}


def kernel(**inputs):
    global last_results
    mask = np.asarray(inputs["mask"])
    max_sc = int(max((mask[b] != 0).sum() for b in range(B)))
    skc = max(P, -(-max_sc // P) * P)
    if skc not in _nc_cache:
        _nc_cache[skc] = build(skc)
    nc = _nc_cache[skc]
    in_maps = _shard(inputs, skc)
    trace = bool(int(os.environ.get("KERNEL_TRACE", "0")))
    if trace:
        _install_ntff_hook()
        import concourse.bass_utils as _bu
        _bu.upload_artifacts = lambda tmpdir: tmpdir
    res = run_bass_kernel_spmd(nc, in_maps, core_ids=list(range(NCORES)),
                               trace=trace)
    last_results = res
    out = np.empty((B, S, D), dtype=np.float32)
    for c in range(NCORES):
        b, g = divmod(c, GROUPS)
        out[b, :, DSL * g:DSL * (g + 1)] = res.results[c]["out"]
    return out


if __name__ == "__main__":
    nc = build()
    print("build ok; instructions:", sum(1 for _ in nc.m.functions[0].basicblocks for _ in _.instructions) if hasattr(nc.m.functions[0], "basicblocks") else "?")
